# revision 19
# baseline (speedup 1.0000x reference)
"""AMICO ADMM solver on 8 Trainium2 NeuronCores.

Problem: X = argmin ||Y^T - A x||^2 + lam*||x||_1 s.t. x >= 0, solved with
max_iter ADMM steps (rho=1, lam=0.1) exactly as in the reference scan.

Algebraic reduction (tracking only v = x + u):
    v_1 = G                      with G  = Minv @ A^T @ Y^T
    for i = 2..N:
        w   = |v - t|            (t = lam/rho)
        S   = min(v, t) + Gb     (Gb = G + cn,  cn = -t * Minv @ 1)
        v'  = Minv @ w + S
    output x_N = Minv @ w_{N-1} + Gb

since z = relu(v - t), u' = v - z = min(v, t), and z - u' = |v - t| - t.

PSUM bias invariant: the first iteration's matmul group includes a rank-1
matmul (cn-row x ones-row) so PSUM holds ps_1 = v_1 + cn = Gb.  The
in-place S-fold  ps <- min(ps, t+cn) + Gb  = S + cn  keeps the invariant
ps_i = v_i + cn for every iteration; the next iteration's matmuls simply
accumulate on top (start=False; PSUM has_written bits persist - only
first_mm=1 clears them).  At iteration 1, Gb == ps so the fold reads its
own bank (no gb16 dependency on the critical startup path); gb16 (an fp16
copy of ps_1) is captured on the DVE off the critical path and serves
as the fold's second operand from iteration 3 on; iteration 2 instead
accumulates onto the untouched ps_1 and restores the skipped S-part
min(ps_1, t+cn) through an identity matmul.  Iteration N-1 resets
the banks to Gb so the final accumulation produces ps_N = Gb + Minv w = x
directly (plain fp16 ACT copy to the output tiles).

Biases:  w = |v - t| = Abs(ps + (-t - cn))   (per-partition ACT bias)

Sharding: data-parallel over voxels (B=4096 -> 512 per core); A-derived
matrices (Minv, Ht) replicated; no cross-core communication.

Performance notes (measured on silicon):
 - fp16 matmuls (fp8/bf16 fail the accuracy budget over 30 non-converged
   iterations; verified by simulation).  16 matmuls/iteration.
 - Matmul emission interleaves the four PSUM-bank groups so each bank's
   stop lands early enough for its Abs/S-fold to complete before the next
   iteration's first matmul on that bank - no PE stalls in steady state.
 - Input DMAs only on the two hardware DGE rings (sync + scalar), ordered
   to match first-use; Minv is packed by output-chunk (miM) so iteration 2
   consumes it in DMA arrival order.  The gpsimd software-DGE ring is ~3x
   slower and is avoided for bulk data.
 - ~6 dummy matmuls on scratch warm the PE HAM clock gate during the DMA
   wait.
"""

import numpy as np

B_VOX = 4096
M_MEAS = 256
K_ATOMS = 512
P = 128
N_CORES = 8
BS = B_VOX // N_CORES  # 512 voxels per core
KB = K_ATOMS // P  # 4 chunks of the contraction/output dim
LAM = 0.1
RHO = 1.0
THR = LAM / RHO

_NC_CACHE = {}

# packed layout offsets (fp16 elements per partition row)
O_HT0 = 0
O_YT0 = 512
O_HT1 = 1024
O_YT1 = 1536
O_MI = 2048              # miM: KB chunks of K_ATOMS, grouped by output chunk
O_CONST = O_MI + KB * K_ATOMS   # 4096: [nb2|tc|ncn|pad (16) | Id (128)]
O_CNONE = O_CONST + 144  # 4240: row 0 only: [cn-row (512) | ones (512)]
NPACK = O_CNONE + K_ATOMS + BS  # 5264

# interleaved matmul emission order for a full 4x4 iteration:
# chunk m's accumulation group stops early enough that its Abs/S-fold
# finish before the next iteration needs the bank, while the last-produced
# w (chunk 3) is consumed as late as possible.
_MM_ORDER = [
    (0, 0), (0, 1), (0, 2),
    (1, 0), (1, 1), (1, 2),
    (0, 3),
    (2, 0), (2, 1), (2, 2),
    (1, 3),
    (3, 0), (3, 1), (3, 2),
    (2, 3),
    (3, 3),
]
# iteration-2 variant: a fifth matmul per group (I @ mn_1) supplies the
# S-part skipped at iteration 1, so iteration 1 needs no PSUM rewrite
_MM_ORDER5 = [
    (0, 0), (0, 1), (0, 2),
    (1, 0), (1, 1), (1, 2),
    (0, 3), (0, "id"),
    (2, 0), (2, 1), (2, 2),
    (1, 3), (1, "id"),
    (3, 0), (3, 1), (3, 2),
    (2, 3), (2, "id"),
    (3, 3), (3, "id"),
]
# G iteration: (m, kb) over kb in {0,1} plus cn rank-1 matmuls (m, 'cn')
_G_ORDER = [
    (0, 0), (1, 0), (0, "cn"),
    (2, 0), (0, 1), (1, "cn"),
    (3, 0), (1, 1), (2, "cn"),
    (2, 1), (3, "cn"), (3, 1),
]


def _build(niter):
    import concourse.mybir as mybir
    import concourse.tile as tile
    from concourse import bacc

    f32 = mybir.dt.float32
    f16 = mybir.dt.float16
    Alu = mybir.AluOpType
    Act = mybir.ActivationFunctionType

    nc = bacc.Bacc(None, target_bir_lowering=False)
    packed = nc.declare_dram_parameter("packed", [P, NPACK], f16, isOutput=False)
    out = nc.declare_dram_parameter("out", [K_ATOMS, BS], f16, isOutput=True)

    with tile.TileContext(nc) as tc:
        with (
            tc.tile_pool(name="const", bufs=1) as cpool,
            tc.tile_pool(name="w", bufs=8) as wpool,
            tc.tile_pool(name="mn", bufs=1) as mpool,
            tc.tile_pool(name="o", bufs=4) as opool,
            tc.tile_pool(name="psum", bufs=1, space="PSUM") as ppool,
            tc.tile_pool(name="pwarm", bufs=1, space="PSUM") as ppwarm,
        ):
            # ---- PE warm-up on scratch (no data deps) ----
            sc_w = cpool.tile([P, P], f16)
            sc_r = cpool.tile([P, BS], f16)
            pwarm = ppwarm.tile([P, BS], f32)
            nc.vector.memset(sc_w[:], 0.0)
            nc.vector.memset(sc_r[:], 0.0)
            for _ in range(6):
                nc.tensor.matmul(pwarm[:], lhsT=sc_w[:], rhs=sc_r[:],
                                 start=True, stop=True)

            # ---- input DMAs: two HWDGE rings, first-use order ----
            hy_sb = cpool.tile([P, 2 * (K_ATOMS + BS)], f16)
            mi_sb = cpool.tile([P, KB * K_ATOMS], f16)
            ct_sb = cpool.tile([P, 144], f16)
            cno_sb = cpool.tile([1, K_ATOMS + BS], f16)

            nc.sync.dma_start(cno_sb[:], packed[0:1, O_CNONE:NPACK])
            nc.sync.dma_start(ct_sb[:], packed[:, O_CONST:O_CNONE])
            nc.sync.dma_start(hy_sb[:, O_YT0:O_HT1], packed[:, O_YT0:O_HT1])
            nc.scalar.dma_start(hy_sb[:, O_HT0:O_YT0], packed[:, O_HT0:O_YT0])
            nc.sync.dma_start(hy_sb[:, O_HT1:O_YT1], packed[:, O_HT1:O_YT1])
            nc.scalar.dma_start(hy_sb[:, O_YT1:2048], packed[:, O_YT1:2048])
            nc.sync.dma_start(mi_sb[:, 0:512], packed[:, O_MI : O_MI + 512])
            nc.scalar.dma_start(mi_sb[:, 512:1024],
                                packed[:, O_MI + 512 : O_MI + 1024])
            nc.sync.dma_start(mi_sb[:, 1024:1536],
                              packed[:, O_MI + 1024 : O_MI + 1536])
            nc.scalar.dma_start(mi_sb[:, 1536:2048],
                                packed[:, O_MI + 1536 : O_MI + 2048])

            # f32 per-partition constants for biases/scalars
            nb2_sb = cpool.tile([P, KB], f32)   # -t - cn
            tc_sb = cpool.tile([P, KB], f32)    # t + cn
            ncn_sb = cpool.tile([P, KB], f32)   # -cn
            nc.vector.tensor_copy(nb2_sb[:], ct_sb[:, 0:4])
            nc.vector.tensor_copy(tc_sb[:], ct_sb[:, 4:8])
            nc.vector.tensor_copy(ncn_sb[:], ct_sb[:, 8:12])

            outr = out.rearrange("(mb p) n -> p mb n", p=P)

            # persistent PSUM: one bank per output chunk, ps = v + cn
            ps = [ppool.tile([P, BS], f32, name=f"ps{m}") for m in range(KB)]

            _kbw = K_ATOMS + BS

            def g_mm(m, kb):
                if kb == "cn":
                    nc.tensor.matmul(
                        ps[m][:],
                        lhsT=cno_sb[:, m * P : (m + 1) * P],
                        rhs=cno_sb[:, K_ATOMS:],
                        start=False, stop=True,
                    )
                else:
                    nc.tensor.matmul(
                        ps[m][:],
                        lhsT=hy_sb[:, kb * _kbw + m * P : kb * _kbw + (m + 1) * P],
                        rhs=hy_sb[:, kb * _kbw + K_ATOMS : (kb + 1) * _kbw],
                        start=(kb == 0), stop=False,
                    )

            id_sb = ct_sb[:, 16:144]  # 128x128 identity (fp16)

            def mi_mm(m, kb, stop):
                nc.tensor.matmul(
                    ps[m][:],
                    lhsT=mi_sb[:, m * K_ATOMS + kb * P : m * K_ATOMS + (kb + 1) * P],
                    rhs=w_cur[kb][:],
                    start=False, stop=stop,
                )

            gb16 = cpool.tile([P, KB, BS], f16)  # Gb, captured from ps_1

            def emit_post(m, it):
                """elementwise ops for chunk m after its group stops at
                iteration `it` (2..niter-1): w-Abs, then S-fold (or the Gb
                reset before the final accumulation)."""
                wm = wpool.tile([P, BS], f16, tag="w", name=f"w{it}_{m}")
                nc.scalar.activation(wm[:], ps[m][:], Act.Abs,
                                     bias=nb2_sb[:, m : m + 1])
                w_new[m] = wm
                if it == niter - 1:
                    # final accumulation target is Gb (ps_N = Gb + Minv w = x)
                    nc.vector.tensor_copy(ps[m][:], gb16[:, m, :])
                else:
                    # S-fold in place: ps <- min(ps, t+cn) + Gb  (= S + cn)
                    nc.vector.scalar_tensor_tensor(
                        ps[m][:], ps[m][:], tc_sb[:, m : m + 1], gb16[:, m, :],
                        Alu.min, Alu.add,
                    )

            w_cur = [None] * KB
            w_new = [None] * KB
            mn_t = [None] * KB

            # ---- iteration 1: ps = G + cn ----
            for m, kb in _G_ORDER:
                g_mm(m, kb)
            if niter == 1:
                for m in range(KB):
                    xm = opool.tile([P, BS], f16, tag="x", name=f"x1{m}")
                    nc.scalar.activation(xm[:], ps[m][:], Act.Copy,
                                         bias=ncn_sb[:, m : m + 1])
                    (nc.sync if m % 2 == 0 else nc.scalar).dma_start(
                        outr[:, m, :], xm[:]
                    )
            else:
                for m in range(KB):
                    # w1 first: Abs is the next-iteration gate
                    wm = wpool.tile([P, BS], f16, tag="w", name=f"w1_{m}")
                    nc.scalar.activation(wm[:], ps[m][:], Act.Abs,
                                         bias=nb2_sb[:, m : m + 1])
                    w_new[m] = wm
                if niter > 2:
                    # iteration 1 leaves PSUM untouched (= Gb); gb16 captures
                    # and the skipped S-part mn = min(ps, t+cn) are pure DVE
                    # reads, off the next iteration's critical path.
                    # Iteration 2 adds mn back via an identity matmul.
                    for m in range(KB):
                        nc.vector.tensor_copy(gb16[:, m, :], ps[m][:])
                    for m in range(KB):
                        mn_m = mpool.tile([P, BS], f16, tag=f"mn{m}",
                                          name=f"mn{m}")
                        nc.vector.tensor_scalar(mn_m[:], ps[m][:],
                                                tc_sb[:, m : m + 1], None,
                                                Alu.min)
                        mn_t[m] = mn_m
                # niter == 2: ps already holds Gb - the final iteration
                # accumulates straight on top
                w_cur, w_new = w_new, [None] * KB

                # ---- iterations 2..niter ----
                for it in range(2, niter + 1):
                    five = (it == 2 and niter > 2)
                    order = _MM_ORDER5 if five else _MM_ORDER
                    last = it == niter
                    for m, kb in order:
                        if kb == "id":
                            nc.tensor.matmul(ps[m][:], lhsT=id_sb,
                                             rhs=mn_t[m][:],
                                             start=False, stop=True)
                        else:
                            mi_mm(m, kb, stop=(kb == KB - 1 and not five))
                        group_done = (kb == "id") or (kb == KB - 1 and not five)
                        if not group_done:
                            continue
                        if last:
                            # final: ps_N = Gb + Minv w = x, straight to DRAM
                            xm = opool.tile([P, BS], f16, tag="x",
                                            name=f"x{m}")
                            nc.scalar.activation(xm[:], ps[m][:], Act.Copy)
                            (nc.sync if m % 2 == 0 else nc.scalar).dma_start(
                                outr[:, m, :], xm[:]
                            )
                        else:
                            emit_post(m, it)
                    if not last:
                        w_cur, w_new = w_new, [None] * KB

    nc.finalize()
    return nc


def _get_nc(niter):
    if niter not in _NC_CACHE:
        _NC_CACHE[niter] = _build(niter)
    return _NC_CACHE[niter]


def _prep_in_maps(Y, A):
    """Host precompute of the A-derived (voxel-independent) factor matrices,
    in float64: the inverse replaces the reference's Cholesky solve. Shards Y
    over voxels (transposed) and packs all device inputs into one
    pre-transposed [128, NPACK] fp16 array."""
    A64 = A.astype(np.float64)
    LHS = A64.T @ A64 + RHO * np.eye(K_ATOMS)
    Minv = np.linalg.inv(LHS)
    Minv = (Minv + Minv.T) / 2
    Hm = A64 @ Minv  # [M, K]
    cn = -THR * Minv.sum(axis=1)  # [K]

    Ht = Hm.astype(np.float16)  # [M, K], M = 2*P exactly
    htp = Ht.reshape(2, P, K_ATOMS).transpose(1, 0, 2)  # [P, 2, K]
    # miM layout: [p, m, kb, c] = Minv[kb*P + p, m*P + c]
    miM = (
        Minv.astype(np.float16)
        .reshape(KB, P, KB, P)
        .transpose(1, 2, 0, 3)
        .reshape(P, KB * K_ATOMS)
    )
    cnq = cn.astype(np.float16).astype(np.float64)  # the cn the device sees
    nb2 = (-THR - cnq).astype(np.float16).reshape(KB, P).T  # [P, KB]
    tcc = (THR + cnq).astype(np.float16).reshape(KB, P).T
    ncn = (-cnq).astype(np.float16).reshape(KB, P).T
    consts = np.concatenate(
        [nb2, tcc, ncn, np.zeros((P, 4), np.float16),
         np.eye(P, dtype=np.float16)], axis=1
    )  # [P, 144]

    cnone = np.zeros((P, K_ATOMS + BS), np.float16)
    cnone[0, :K_ATOMS] = cn.astype(np.float16)
    cnone[0, K_ATOMS:] = 1.0

    in_maps = []
    for c in range(N_CORES):
        Yt = Y[c * BS : (c + 1) * BS, :].T.astype(np.float16)  # [M, BS]
        ytp = Yt.reshape(2, P, BS).transpose(1, 0, 2)  # [P, 2, BS]
        hy = np.concatenate([htp, ytp], axis=2).reshape(P, 2 * (K_ATOMS + BS))
        pk = np.ascontiguousarray(
            np.concatenate([hy, miM, consts, cnone], axis=1)
        )
        in_maps.append({"packed": pk})
    return in_maps


def kernel(Y, A, max_iter):
    from concourse.bass_utils import run_bass_kernel_spmd

    Y = np.ascontiguousarray(np.asarray(Y, dtype=np.float32))
    A = np.ascontiguousarray(np.asarray(A, dtype=np.float32))
    niter = int(max_iter)
    assert Y.shape == (B_VOX, M_MEAS) and A.shape == (M_MEAS, K_ATOMS)
    if niter < 1:
        # zero-length scan returns the zero initial state
        return np.zeros((B_VOX, K_ATOMS), np.float32)

    in_maps = _prep_in_maps(Y, A)
    nc = _get_nc(niter)
    res = run_bass_kernel_spmd(nc, in_maps, core_ids=list(range(N_CORES)))

    outp = np.empty((B_VOX, K_ATOMS), np.float32)
    for c in range(N_CORES):
        outp[c * BS : (c + 1) * BS] = res.results[c]["out"].T.astype(np.float32)
    return outp


# revision 20
# speedup vs baseline: 1.0735x; 1.0735x over previous
"""AMICO ADMM solver on 8 Trainium2 NeuronCores.

Problem: X = argmin ||Y^T - A x||^2 + lam*||x||_1 s.t. x >= 0, solved with
max_iter ADMM steps (rho=1, lam=0.1) exactly as in the reference scan.

Algebraic reduction (tracking only v = x + u):
    v_1 = G                      with G  = Minv @ A^T @ Y^T
    for i = 2..N:
        w   = |v - t|            (t = lam/rho)
        S   = min(v, t) + Gb     (Gb = G - t * Minv @ 1)
        v'  = Minv @ w + S
    output x_N = Minv @ w_{N-1} + Gb

since z = relu(v - t), u' = v - z = min(v, t), and z - u' = |v - t| - t.

Sharding: data-parallel over voxels (B=4096 -> 512 per core); A-derived
matrices (Minv, Ht) replicated; no cross-core communication.

Implementation notes (measured on silicon):
 - All matmul operands are fp16 (fp8/bf16 lose too much accuracy over 30
   non-converged iterations; verified by simulation).
 - In-place S-fold: after each iteration's 4-matmul accumulation group
   fills PSUM bank m with v, ACT reads it for w = |v - t| and then DVE
   rewrites the bank IN PLACE with S' = min(v, t) + Gb.  The next
   iteration's matmuls accumulate onto S' with start=False - the PSUM
   has_written bits persist from the previous matmul group (only
   first_mm=1 clears them), so no identity matmuls are needed.  This
   cuts PE work from 18 to 16 matmuls/iteration and removes the
   separate fp16 v materialization from the DVE.
 - PE warm-up: ~8 dummy matmuls on a scratch PSUM bank issue during the
   input-DMA wait so the HAM clock gate reaches 2.4 GHz before the real
   matmuls start.
 - Input DMAs issue in parallel from the sync, scalar(ACT) and gpsimd
   queues; output DMAs are split across sync/gpsimd, per chunk, so the
   final transfers overlap the last iteration's compute.
"""

import numpy as np

B_VOX = 4096
M_MEAS = 256
K_ATOMS = 512
P = 128
N_CORES = 8
BS = B_VOX // N_CORES  # 512 voxels per core
KB = K_ATOMS // P  # 4 chunks of the contraction/output dim
LAM = 0.1
RHO = 1.0
THR = LAM / RHO

_NC_CACHE = {}

# packed layout offsets (fp16 elements per partition row)
O_HT0 = 0
O_YT0 = 512
O_HY1 = 1024            # Ht1 | Yt1
O_CN = 2048             # cneg [KB]
O_MI = 2052             # Minv, KB chunks of 512
NPACK = O_MI + KB * K_ATOMS  # 4100


def _build(niter):
    import concourse.mybir as mybir
    import concourse.tile as tile
    from concourse import bacc

    f32 = mybir.dt.float32
    f16 = mybir.dt.float16
    Alu = mybir.AluOpType
    Act = mybir.ActivationFunctionType

    nc = bacc.Bacc(None, target_bir_lowering=False)
    packed = nc.declare_dram_parameter("packed", [P, NPACK], f16, isOutput=False)
    out = nc.declare_dram_parameter("out", [K_ATOMS, BS], f16, isOutput=True)

    with tile.TileContext(nc) as tc:
        with (
            tc.tile_pool(name="const", bufs=1) as cpool,
            tc.tile_pool(name="w", bufs=8) as wpool,
            tc.tile_pool(name="o", bufs=4) as opool,
            tc.tile_pool(name="psum", bufs=1, space="PSUM") as ppool,
            tc.tile_pool(name="pwarm", bufs=1, space="PSUM") as ppwarm,
        ):
            # ---- PE warm-up: dummy matmuls on uninitialized scratch ----
            # (no data deps, so they issue right after the preamble barrier
            # and keep the HAM activity window busy during the DMA wait)
            sc_w = cpool.tile([P, P], f16)
            sc_r = cpool.tile([P, BS], f16)
            pwarm = ppwarm.tile([P, BS], f32)
            nc.vector.memset(sc_w[:], 0.0)
            nc.vector.memset(sc_r[:], 0.0)
            for _ in range(8):
                nc.tensor.matmul(pwarm[:], lhsT=sc_w[:], rhs=sc_r[:],
                                 start=True, stop=True)

            nb = cpool.tile([P, 1], f32)
            nc.vector.memset(nb[:], -THR)

            # ---- input DMAs: parallel issue across sync/scalar/gpsimd ----
            hy_sb = cpool.tile([P, 2 * (K_ATOMS + BS)], f16)
            mi_sb = cpool.tile([P, KB + KB * K_ATOMS], f16)  # cneg + Minv
            nc.sync.dma_start(hy_sb[:, 0:512], packed[:, O_HT0:O_YT0])
            nc.scalar.dma_start(hy_sb[:, 512:1024], packed[:, O_YT0:O_HY1])
            nc.sync.dma_start(hy_sb[:, 1024:1536], packed[:, O_HY1 : O_HY1 + 512])
            nc.scalar.dma_start(hy_sb[:, 1536:2048],
                                packed[:, O_HY1 + 512 : O_CN])
            nc.sync.dma_start(mi_sb[:, 0 : KB + 512],
                              packed[:, O_CN : O_CN + KB + 512])
            nc.scalar.dma_start(mi_sb[:, KB + 512 : KB + 1024],
                                packed[:, O_CN + KB + 512 : O_CN + KB + 1024])
            nc.sync.dma_start(mi_sb[:, KB + 1024 : KB + 1536],
                              packed[:, O_CN + KB + 1024 : O_CN + KB + 1536])
            nc.scalar.dma_start(mi_sb[:, KB + 1536 :],
                                packed[:, O_CN + KB + 1536 :])

            cn_sb = cpool.tile([P, KB], f32)
            nc.vector.tensor_copy(cn_sb[:], mi_sb[:, 0:KB])
            gb16 = cpool.tile([P, KB, BS], f16)

            _kbw = K_ATOMS + BS
            MIW = KB  # Minv starts after cneg inside mi_sb

            outr = out.rearrange("(mb p) n -> p mb n", p=P)

            # persistent PSUM: 4 banks, one tile per output chunk
            ps = [ppool.tile([P, BS], f32, name=f"ps{m}") for m in range(KB)]

            w_cur = [None] * KB

            # ---- iteration 1: G = Ht^T @ Yt ----
            for m in range(KB):
                for kb in range(2):
                    nc.tensor.matmul(
                        ps[m][:],
                        lhsT=hy_sb[:, kb * _kbw + m * P : kb * _kbw + (m + 1) * P],
                        rhs=hy_sb[:, kb * _kbw + K_ATOMS : (kb + 1) * _kbw],
                        start=(kb == 0),
                        stop=(kb == 1),
                    )
                if niter == 1:
                    xm = opool.tile([P, BS], f16, tag="x", name=f"x1{m}")
                    nc.scalar.activation(xm[:], ps[m][:], Act.Copy)
                    (nc.sync if m % 2 == 0 else nc.scalar).dma_start(
                        outr[:, m, :], xm[:]
                    )
                    continue
                wm = wpool.tile([P, BS], f16, tag="w", name=f"w1_{m}")
                nc.scalar.activation(wm[:], ps[m][:], Act.Abs, bias=nb[:, 0:1])
                w_cur[m] = wm
                # Gb to SBUF with the -t*rowsum bias folded in
                nc.scalar.activation(
                    gb16[:, m, :], ps[m][:], Act.Identity,
                    bias=cn_sb[:, m : m + 1],
                )
                if niter == 2:
                    nc.vector.tensor_copy(ps[m][:], gb16[:, m, :])
                else:
                    # S1 in place: ps <- min(ps, t) + Gb
                    nc.vector.scalar_tensor_tensor(
                        ps[m][:], ps[m][:], THR, gb16[:, m, :],
                        Alu.min, Alu.add,
                    )

            # ---- iterations 2..niter ----
            for it in range(2, niter + 1):
                last = it == niter
                neww = [None] * KB
                for m in range(KB):
                    for kb in range(KB):
                        nc.tensor.matmul(
                            ps[m][:],
                            lhsT=mi_sb[:, MIW + kb * K_ATOMS + m * P : MIW + kb * K_ATOMS + (m + 1) * P],
                            rhs=w_cur[kb][:],
                            start=False,
                            stop=(kb == KB - 1),
                        )
                    if last:
                        xm = opool.tile([P, BS], f16, tag="x", name=f"x{m}")
                        nc.scalar.activation(xm[:], ps[m][:], Act.Copy)
                        (nc.sync if m % 2 == 0 else nc.scalar).dma_start(
                            outr[:, m, :], xm[:]
                        )
                        continue
                    wm = wpool.tile([P, BS], f16, tag="w", name=f"w{it}_{m}")
                    nc.scalar.activation(wm[:], ps[m][:], Act.Abs, bias=nb[:, 0:1])
                    neww[m] = wm
                    if it == niter - 1:
                        # final accumulation target is Gb, not S'
                        nc.vector.tensor_copy(ps[m][:], gb16[:, m, :])
                    else:
                        nc.vector.scalar_tensor_tensor(
                            ps[m][:], ps[m][:], THR, gb16[:, m, :],
                            Alu.min, Alu.add,
                        )
                if not last:
                    w_cur = neww

    nc.finalize()
    return nc


def _get_nc(niter):
    if niter not in _NC_CACHE:
        _NC_CACHE[niter] = _build(niter)
    return _NC_CACHE[niter]


def _prep_in_maps(Y, A):
    """Host precompute of the A-derived (voxel-independent) factor matrices,
    in float64: the inverse replaces the reference's Cholesky solve. Shards Y
    over voxels (transposed) and packs all device inputs into one
    pre-transposed [128, NPACK] fp16 array so every DMA descriptor is a
    multi-KB contiguous run."""
    A64 = A.astype(np.float64)
    LHS = A64.T @ A64 + RHO * np.eye(K_ATOMS)
    Minv = np.linalg.inv(LHS)
    Minv = (Minv + Minv.T) / 2
    Hm = A64 @ Minv  # [M, K]
    rsum = Minv.sum(axis=1)

    Ht = Hm.astype(np.float16)  # [M, K], M = 2*P exactly
    htp = Ht.reshape(2, P, K_ATOMS).transpose(1, 0, 2)  # [P, 2, K]
    Mi = Minv.astype(np.float16)
    mip = Mi.reshape(KB, P, K_ATOMS).transpose(1, 0, 2).reshape(P, KB * K_ATOMS)
    cneg = (-THR * rsum).astype(np.float16).reshape(KB, P).T  # [P, KB]
    fixed = np.concatenate([cneg, mip], axis=1)  # [P, KB + KB*K]

    in_maps = []
    for c in range(N_CORES):
        Yt = Y[c * BS : (c + 1) * BS, :].T.astype(np.float16)  # [M, BS]
        ytp = Yt.reshape(2, P, BS).transpose(1, 0, 2)  # [P, 2, BS]
        hy = np.concatenate([htp, ytp], axis=2).reshape(P, 2 * (K_ATOMS + BS))
        pk = np.ascontiguousarray(np.concatenate([hy, fixed], axis=1))
        in_maps.append({"packed": pk})
    return in_maps


def kernel(Y, A, max_iter):
    from concourse.bass_utils import run_bass_kernel_spmd

    Y = np.ascontiguousarray(np.asarray(Y, dtype=np.float32))
    A = np.ascontiguousarray(np.asarray(A, dtype=np.float32))
    niter = int(max_iter)
    assert Y.shape == (B_VOX, M_MEAS) and A.shape == (M_MEAS, K_ATOMS)
    if niter < 1:
        # zero-length scan returns the zero initial state
        return np.zeros((B_VOX, K_ATOMS), np.float32)

    in_maps = _prep_in_maps(Y, A)
    nc = _get_nc(niter)
    res = run_bass_kernel_spmd(nc, in_maps, core_ids=list(range(N_CORES)))

    outp = np.empty((B_VOX, K_ATOMS), np.float32)
    for c in range(N_CORES):
        outp[c * BS : (c + 1) * BS] = res.results[c]["out"].T.astype(np.float32)
    return outp


# revision 21
# speedup vs baseline: 1.0748x; 1.0013x over previous
"""AMICO ADMM solver on 8 Trainium2 NeuronCores.

Problem: X = argmin ||Y^T - A x||^2 + lam*||x||_1 s.t. x >= 0, solved with
max_iter ADMM steps (rho=1, lam=0.1) exactly as in the reference scan.

Algebraic reduction (tracking only v = x + u):
    v_1 = G                      with G  = Minv @ A^T @ Y^T
    for i = 2..N:
        w   = |v - t|            (t = lam/rho)
        S   = min(v, t) + Gb     (Gb = G - t * Minv @ 1)
        v'  = Minv @ w + S
    output x_N = Minv @ w_{N-1} + Gb

since z = relu(v - t), u' = v - z = min(v, t), and z - u' = |v - t| - t.

Sharding: data-parallel over voxels (B=4096 -> 512 per core); A-derived
matrices (Minv, Ht) replicated; no cross-core communication.

Implementation notes (measured on silicon):
 - All matmul operands are fp16 (fp8/bf16 lose too much accuracy over 30
   non-converged iterations; verified by simulation).
 - In-place S-fold: after each iteration's 4-matmul accumulation group
   fills PSUM bank m with v, ACT reads it for w = |v - t| and then DVE
   rewrites the bank IN PLACE with S' = min(v, t) + Gb.  The next
   iteration's matmuls accumulate onto S' with start=False - the PSUM
   has_written bits persist from the previous matmul group (only
   first_mm=1 clears them), so no identity matmuls are needed.  This
   cuts PE work from 18 to 16 matmuls/iteration and removes the
   separate fp16 v materialization from the DVE.
 - PE warm-up: ~8 dummy matmuls on a scratch PSUM bank issue during the
   input-DMA wait so the HAM clock gate reaches 2.4 GHz before the real
   matmuls start.
 - Input DMAs issue in parallel from the sync, scalar(ACT) and gpsimd
   queues; output DMAs are split across sync/gpsimd, per chunk, so the
   final transfers overlap the last iteration's compute.
"""

import numpy as np

B_VOX = 4096
M_MEAS = 256
K_ATOMS = 512
P = 128
N_CORES = 8
BS = B_VOX // N_CORES  # 512 voxels per core
KB = K_ATOMS // P  # 4 chunks of the contraction/output dim
LAM = 0.1
RHO = 1.0
THR = LAM / RHO

_NC_CACHE = {}

# packed layout offsets (fp16 elements per partition row)
O_HT0 = 0
O_YT0 = 512
O_HY1 = 1024            # Ht1 | Yt1
O_CN = 2048             # cneg [KB]
O_MI = 2052             # Minv, KB chunks of 512
NPACK = O_MI + KB * K_ATOMS  # 4100


def _build(niter):
    import concourse.mybir as mybir
    import concourse.tile as tile
    from concourse import bacc

    f32 = mybir.dt.float32
    f16 = mybir.dt.float16
    Alu = mybir.AluOpType
    Act = mybir.ActivationFunctionType

    nc = bacc.Bacc(None, target_bir_lowering=False)
    packed = nc.declare_dram_parameter("packed", [P, NPACK], f16, isOutput=False)
    out = nc.declare_dram_parameter("out", [K_ATOMS, BS], f16, isOutput=True)

    with tile.TileContext(nc) as tc:
        with (
            tc.tile_pool(name="const", bufs=1) as cpool,
            tc.tile_pool(name="w", bufs=8) as wpool,
            tc.tile_pool(name="o", bufs=4) as opool,
            tc.tile_pool(name="psum", bufs=1, space="PSUM") as ppool,
            tc.tile_pool(name="pwarm", bufs=1, space="PSUM") as ppwarm,
        ):
            # ---- PE warm-up: dummy matmuls on uninitialized scratch ----
            # (no data deps, so they issue right after the preamble barrier
            # and keep the HAM activity window busy during the DMA wait)
            sc_w = cpool.tile([P, P], f16)
            sc_r = cpool.tile([P, BS], f16)
            pwarm = ppwarm.tile([P, BS], f32)
            nc.vector.memset(sc_w[:], 0.0)
            nc.vector.memset(sc_r[:], 0.0)
            for _ in range(8):
                nc.tensor.matmul(pwarm[:], lhsT=sc_w[:], rhs=sc_r[:],
                                 start=True, stop=True)

            nb = cpool.tile([P, 1], f32)
            nc.vector.memset(nb[:], -THR)

            # ---- input DMAs: parallel issue across sync/scalar/gpsimd ----
            hy_sb = cpool.tile([P, 2 * (K_ATOMS + BS)], f16)
            mi_sb = cpool.tile([P, KB + KB * K_ATOMS], f16)  # cneg + Minv
            nc.sync.dma_start(hy_sb[:, 0:512], packed[:, O_HT0:O_YT0])
            nc.scalar.dma_start(hy_sb[:, 512:1024], packed[:, O_YT0:O_HY1])
            nc.sync.dma_start(hy_sb[:, 1024:1536], packed[:, O_HY1 : O_HY1 + 512])
            nc.scalar.dma_start(hy_sb[:, 1536:2048],
                                packed[:, O_HY1 + 512 : O_CN])
            nc.sync.dma_start(mi_sb[:, 0 : KB + 512],
                              packed[:, O_CN : O_CN + KB + 512])
            nc.scalar.dma_start(mi_sb[:, KB + 512 : KB + 1024],
                                packed[:, O_CN + KB + 512 : O_CN + KB + 1024])
            nc.sync.dma_start(mi_sb[:, KB + 1024 : KB + 1536],
                              packed[:, O_CN + KB + 1024 : O_CN + KB + 1536])
            nc.scalar.dma_start(mi_sb[:, KB + 1536 :],
                                packed[:, O_CN + KB + 1536 :])

            cn_sb = cpool.tile([P, KB], f32)
            nc.vector.tensor_copy(cn_sb[:], mi_sb[:, 0:KB])
            gb16 = cpool.tile([P, KB, BS], f16)

            _kbw = K_ATOMS + BS
            MIW = KB  # Minv starts after cneg inside mi_sb

            outr = out.rearrange("(mb p) n -> p mb n", p=P)

            # persistent PSUM: 4 banks, one tile per output chunk
            ps = [ppool.tile([P, BS], f32, name=f"ps{m}") for m in range(KB)]

            w_cur = [None] * KB

            # ---- iteration 1: G = Ht^T @ Yt ----
            for m in range(KB):
                for kb in range(2):
                    nc.tensor.matmul(
                        ps[m][:],
                        lhsT=hy_sb[:, kb * _kbw + m * P : kb * _kbw + (m + 1) * P],
                        rhs=hy_sb[:, kb * _kbw + K_ATOMS : (kb + 1) * _kbw],
                        start=(kb == 0),
                        stop=(kb == 1),
                    )
                if niter == 1:
                    xm = opool.tile([P, BS], f16, tag="x", name=f"x1{m}")
                    nc.scalar.activation(xm[:], ps[m][:], Act.Copy)
                    (nc.sync if m % 2 == 0 else nc.scalar).dma_start(
                        outr[:, m, :], xm[:]
                    )
                    continue
                wm = wpool.tile([P, BS], f16, tag="w", name=f"w1_{m}")
                nc.scalar.activation(wm[:], ps[m][:], Act.Abs, bias=nb[:, 0:1])
                w_cur[m] = wm
            if niter >= 2:
                # Gb = ps + cn: chunks 0,1 on DVE (tensor_scalar add), 2,3 on
                # ACT after the Abs ops - halves the iter-1 ACT chain
                def _g(m):
                    if m < 2:
                        nc.vector.tensor_scalar(gb16[:, m, :], ps[m][:],
                                                cn_sb[:, m : m + 1], None,
                                                Alu.add)
                    else:
                        nc.scalar.activation(gb16[:, m, :], ps[m][:],
                                             Act.Identity,
                                             bias=cn_sb[:, m : m + 1])

                def _f(m):
                    if niter == 2:
                        nc.vector.tensor_copy(ps[m][:], gb16[:, m, :])
                    else:
                        # S1 in place: ps <- min(ps, t) + Gb
                        nc.vector.scalar_tensor_tensor(
                            ps[m][:], ps[m][:], THR, gb16[:, m, :],
                            Alu.min, Alu.add,
                        )

                _g(0); _f(0); _g(1); _f(1)
                _g(2); _g(3); _f(2); _f(3)

            # ---- iterations 2..niter ----
            for it in range(2, niter + 1):
                last = it == niter
                neww = [None] * KB
                ORD = [(0, 0), (0, 1), (0, 2), (1, 0), (0, 3),
                       (1, 1), (1, 2), (1, 3), (2, 0), (2, 1), (2, 2), (2, 3),
                       (3, 0), (3, 1), (3, 2), (3, 3)]
                for m, kb in ORD:
                    nc.tensor.matmul(
                        ps[m][:],
                        lhsT=mi_sb[:, MIW + kb * K_ATOMS + m * P : MIW + kb * K_ATOMS + (m + 1) * P],
                        rhs=w_cur[kb][:],
                        start=False,
                        stop=(kb == KB - 1),
                    )
                    if kb != KB - 1:
                        continue
                    if last:
                        xm = opool.tile([P, BS], f16, tag="x", name=f"x{m}")
                        if m == KB - 1:
                            H = BS // 2
                            nc.scalar.activation(xm[:, 0:H], ps[m][:, 0:H],
                                                 Act.Copy)
                            nc.sync.dma_start(outr[:, m, 0:H], xm[:, 0:H])
                            nc.scalar.activation(xm[:, H:], ps[m][:, H:],
                                                 Act.Copy)
                            nc.scalar.dma_start(outr[:, m, H:], xm[:, H:])
                        else:
                            nc.scalar.activation(xm[:], ps[m][:], Act.Copy)
                            (nc.sync if m % 2 == 0 else nc.scalar).dma_start(
                                outr[:, m, :], xm[:]
                            )
                        continue
                    wm = wpool.tile([P, BS], f16, tag="w", name=f"w{it}_{m}")
                    nc.scalar.activation(wm[:], ps[m][:], Act.Abs, bias=nb[:, 0:1])
                    neww[m] = wm
                    if it == niter - 1:
                        # final accumulation target is Gb, not S'
                        nc.vector.tensor_copy(ps[m][:], gb16[:, m, :])
                    else:
                        nc.vector.scalar_tensor_tensor(
                            ps[m][:], ps[m][:], THR, gb16[:, m, :],
                            Alu.min, Alu.add,
                        )
                if not last:
                    w_cur = neww

    nc.finalize()
    return nc


def _get_nc(niter):
    if niter not in _NC_CACHE:
        _NC_CACHE[niter] = _build(niter)
    return _NC_CACHE[niter]


def _prep_in_maps(Y, A):
    """Host precompute of the A-derived (voxel-independent) factor matrices,
    in float64: the inverse replaces the reference's Cholesky solve. Shards Y
    over voxels (transposed) and packs all device inputs into one
    pre-transposed [128, NPACK] fp16 array so every DMA descriptor is a
    multi-KB contiguous run."""
    A64 = A.astype(np.float64)
    LHS = A64.T @ A64 + RHO * np.eye(K_ATOMS)
    Minv = np.linalg.inv(LHS)
    Minv = (Minv + Minv.T) / 2
    Hm = A64 @ Minv  # [M, K]
    rsum = Minv.sum(axis=1)

    Ht = Hm.astype(np.float16)  # [M, K], M = 2*P exactly
    htp = Ht.reshape(2, P, K_ATOMS).transpose(1, 0, 2)  # [P, 2, K]
    Mi = Minv.astype(np.float16)
    mip = Mi.reshape(KB, P, K_ATOMS).transpose(1, 0, 2).reshape(P, KB * K_ATOMS)
    cneg = (-THR * rsum).astype(np.float16).reshape(KB, P).T  # [P, KB]
    fixed = np.concatenate([cneg, mip], axis=1)  # [P, KB + KB*K]

    in_maps = []
    for c in range(N_CORES):
        Yt = Y[c * BS : (c + 1) * BS, :].T.astype(np.float16)  # [M, BS]
        ytp = Yt.reshape(2, P, BS).transpose(1, 0, 2)  # [P, 2, BS]
        hy = np.concatenate([htp, ytp], axis=2).reshape(P, 2 * (K_ATOMS + BS))
        pk = np.ascontiguousarray(np.concatenate([hy, fixed], axis=1))
        in_maps.append({"packed": pk})
    return in_maps


def kernel(Y, A, max_iter):
    from concourse.bass_utils import run_bass_kernel_spmd

    Y = np.ascontiguousarray(np.asarray(Y, dtype=np.float32))
    A = np.ascontiguousarray(np.asarray(A, dtype=np.float32))
    niter = int(max_iter)
    assert Y.shape == (B_VOX, M_MEAS) and A.shape == (M_MEAS, K_ATOMS)
    if niter < 1:
        # zero-length scan returns the zero initial state
        return np.zeros((B_VOX, K_ATOMS), np.float32)

    in_maps = _prep_in_maps(Y, A)
    nc = _get_nc(niter)
    res = run_bass_kernel_spmd(nc, in_maps, core_ids=list(range(N_CORES)))

    outp = np.empty((B_VOX, K_ATOMS), np.float32)
    for c in range(N_CORES):
        outp[c * BS : (c + 1) * BS] = res.results[c]["out"].T.astype(np.float32)
    return outp


# revision 22
# speedup vs baseline: 1.1033x; 1.0265x over previous
"""AMICO ADMM solver on 8 Trainium2 NeuronCores.

Problem: X = argmin ||Y^T - A x||^2 + lam*||x||_1 s.t. x >= 0, solved with
max_iter ADMM steps (rho=1, lam=0.1) exactly as in the reference scan.

Algebraic reduction (tracking only v = x + u):
    v_1 = G                      with G  = Minv @ A^T @ Y^T
    for i = 2..N:
        w   = |v - t|            (t = lam/rho)
        S   = min(v, t) + Gb     (Gb = G - t * Minv @ 1)
        v'  = Minv @ w + S
    output x_N = Minv @ w_{N-1} + Gb

since z = relu(v - t), u' = v - z = min(v, t), and z - u' = |v - t| - t.

Sharding: data-parallel over voxels (B=4096 -> 512 per core); A-derived
matrices (Minv, Ht) replicated; no cross-core communication.

Implementation notes (measured on silicon):
 - All matmul operands are fp16 (fp8/bf16 lose too much accuracy over 30
   non-converged iterations; verified by simulation).
 - In-place S-fold: after each iteration's 4-matmul accumulation group
   fills PSUM bank m with v, ACT reads it for w = |v - t| and then DVE
   rewrites the bank IN PLACE with S' = min(v, t) + Gb.  The next
   iteration's matmuls accumulate onto S' with start=False - the PSUM
   has_written bits persist from the previous matmul group (only
   first_mm=1 clears them), so no identity matmuls are needed.  This
   cuts PE work from 18 to 16 matmuls/iteration and removes the
   separate fp16 v materialization from the DVE.
 - PE warm-up: ~8 dummy matmuls on a scratch PSUM bank issue during the
   input-DMA wait so the HAM clock gate reaches 2.4 GHz before the real
   matmuls start.
 - Input DMAs issue in parallel from the sync, scalar(ACT) and gpsimd
   queues; output DMAs are split across sync/gpsimd, per chunk, so the
   final transfers overlap the last iteration's compute.
"""

import numpy as np

B_VOX = 4096
M_MEAS = 256
K_ATOMS = 512
P = 128
N_CORES = 8
BS = B_VOX // N_CORES  # 512 voxels per core
KB = K_ATOMS // P  # 4 chunks of the contraction/output dim
LAM = 0.1
RHO = 1.0
THR = LAM / RHO

_NC_CACHE = {}

# packed layout offsets (fp16 elements per partition row)
O_HT0 = 0
O_YT0 = 512
O_HY1 = 1024            # Ht1 | Yt1
O_CN = 2048             # cneg [KB]
O_MI = 2052             # Minv, KB chunks of 512
NPACK = O_MI + KB * K_ATOMS  # 4100


def _build(niter):
    import concourse.mybir as mybir
    import concourse.tile as tile
    from concourse import bacc

    f32 = mybir.dt.float32
    f16 = mybir.dt.float16
    Alu = mybir.AluOpType
    Act = mybir.ActivationFunctionType

    nc = bacc.Bacc(None, target_bir_lowering=False)
    packed = nc.declare_dram_parameter("packed", [P, NPACK], f16, isOutput=False)
    out = nc.declare_dram_parameter("out", [K_ATOMS, BS], f16, isOutput=True)

    with tile.TileContext(nc) as tc:
        with (
            tc.tile_pool(name="const", bufs=1) as cpool,
            tc.tile_pool(name="w", bufs=8) as wpool,
            tc.tile_pool(name="o", bufs=4) as opool,
            tc.tile_pool(name="psum", bufs=1, space="PSUM") as ppool,
            tc.tile_pool(name="pwarm", bufs=1, space="PSUM") as ppwarm,
        ):
            # ---- PE warm-up: dummy matmuls on uninitialized scratch ----
            # (no data deps, so they issue right after the preamble barrier
            # and keep the HAM activity window busy during the DMA wait)
            sc_w = cpool.tile([P, P], f16)
            sc_r = cpool.tile([P, BS], f16)
            pwarm = ppwarm.tile([P, BS], f32)
            nc.vector.memset(sc_w[:], 0.0)
            nc.vector.memset(sc_r[:], 0.0)
            for _ in range(8):
                nc.tensor.matmul(pwarm[:], lhsT=sc_w[:], rhs=sc_r[:],
                                 start=True, stop=True)

            nb = cpool.tile([P, 1], f32)
            nc.vector.memset(nb[:], -THR)

            # ---- input DMAs: parallel issue across sync/scalar/gpsimd ----
            hy_sb = cpool.tile([P, 2 * (K_ATOMS + BS)], f16)
            mi_sb = cpool.tile([P, KB + KB * K_ATOMS], f16)  # cneg + Minv
            nc.sync.dma_start(hy_sb[:, 0:512], packed[:, O_HT0:O_YT0])
            nc.scalar.dma_start(hy_sb[:, 512:1024], packed[:, O_YT0:O_HY1])
            nc.sync.dma_start(hy_sb[:, 1024:1536], packed[:, O_HY1 : O_HY1 + 512])
            nc.scalar.dma_start(hy_sb[:, 1536:2048],
                                packed[:, O_HY1 + 512 : O_CN])
            nc.sync.dma_start(mi_sb[:, 0 : KB + 512],
                              packed[:, O_CN : O_CN + KB + 512])
            nc.scalar.dma_start(mi_sb[:, KB + 512 : KB + 1024],
                                packed[:, O_CN + KB + 512 : O_CN + KB + 1024])
            nc.sync.dma_start(mi_sb[:, KB + 1024 : KB + 1536],
                              packed[:, O_CN + KB + 1024 : O_CN + KB + 1536])
            nc.scalar.dma_start(mi_sb[:, KB + 1536 :],
                                packed[:, O_CN + KB + 1536 :])

            cn_sb = cpool.tile([P, KB], f32)
            nc.vector.tensor_copy(cn_sb[:], mi_sb[:, 0:KB])
            gb16 = cpool.tile([P, KB, BS], f16)

            _kbw = K_ATOMS + BS
            MIW = KB  # Minv starts after cneg inside mi_sb

            outr = out.rearrange("(mb p) n -> p mb n", p=P)

            # persistent PSUM: 4 banks, one tile per output chunk
            ps = [ppool.tile([P, BS], f32, name=f"ps{m}") for m in range(KB)]

            w_cur = [None] * KB

            # ---- iteration 1: G = Ht^T @ Yt ----
            for m in range(KB):
                for kb in range(2):
                    nc.tensor.matmul(
                        ps[m][:],
                        lhsT=hy_sb[:, kb * _kbw + m * P : kb * _kbw + (m + 1) * P],
                        rhs=hy_sb[:, kb * _kbw + K_ATOMS : (kb + 1) * _kbw],
                        start=(kb == 0),
                        stop=(kb == 1),
                    )
                if niter == 1:
                    xm = opool.tile([P, BS], f16, tag="x", name=f"x1{m}")
                    nc.scalar.activation(xm[:], ps[m][:], Act.Copy)
                    (nc.sync if m % 2 == 0 else nc.scalar).dma_start(
                        outr[:, m, :], xm[:]
                    )
                    continue
                wm = wpool.tile([P, BS], f16, tag="w", name=f"w1_{m}")
                nc.scalar.activation(wm[:], ps[m][:], Act.Abs, bias=nb[:, 0:1])
                w_cur[m] = wm
            if niter >= 2:
                # Gb = ps + cn: chunks 0,1 on DVE (tensor_scalar add), 2,3 on
                # ACT after the Abs ops - halves the iter-1 ACT chain
                def _g(m):
                    if m < 2:
                        nc.vector.tensor_scalar(gb16[:, m, :], ps[m][:],
                                                cn_sb[:, m : m + 1], None,
                                                Alu.add)
                    else:
                        nc.scalar.activation(gb16[:, m, :], ps[m][:],
                                             Act.Identity,
                                             bias=cn_sb[:, m : m + 1])

                def _f(m):
                    if niter == 2:
                        nc.vector.tensor_copy(ps[m][:], gb16[:, m, :])
                    else:
                        # S1 in place: ps <- min(ps, t) + Gb
                        nc.vector.scalar_tensor_tensor(
                            ps[m][:], ps[m][:], THR, gb16[:, m, :],
                            Alu.min, Alu.add,
                        )

                _g(0); _f(0); _g(1); _f(1)
                _g(2); _g(3); _f(2); _f(3)

            # ---- iterations 2..niter ----
            for it in range(2, niter + 1):
                last = it == niter
                neww = [None] * KB
                ORD = [(0, 0), (0, 1), (0, 2), (1, 0), (1, 1), (0, 3),
                       (1, 2), (1, 3), (2, 0), (2, 1), (2, 2), (2, 3),
                       (3, 0), (3, 1), (3, 2), (3, 3)]
                for m, kb in ORD:
                    nc.tensor.matmul(
                        ps[m][:],
                        lhsT=mi_sb[:, MIW + kb * K_ATOMS + m * P : MIW + kb * K_ATOMS + (m + 1) * P],
                        rhs=w_cur[kb][:],
                        start=False,
                        stop=(kb == KB - 1),
                    )
                    if kb != KB - 1:
                        continue
                    if last:
                        xm = opool.tile([P, BS], f16, tag="x", name=f"x{m}")
                        if m == KB - 1:
                            H = BS // 2
                            nc.scalar.activation(xm[:, 0:H], ps[m][:, 0:H],
                                                 Act.Copy)
                            nc.sync.dma_start(outr[:, m, 0:H], xm[:, 0:H])
                            nc.scalar.activation(xm[:, H:], ps[m][:, H:],
                                                 Act.Copy)
                            nc.scalar.dma_start(outr[:, m, H:], xm[:, H:])
                        else:
                            nc.scalar.activation(xm[:], ps[m][:], Act.Copy)
                            (nc.sync if m % 2 == 0 else nc.scalar).dma_start(
                                outr[:, m, :], xm[:]
                            )
                        continue
                    wm = wpool.tile([P, BS], f16, tag="w", name=f"w{it}_{m}")
                    nc.scalar.activation(wm[:], ps[m][:], Act.Abs, bias=nb[:, 0:1])
                    neww[m] = wm
                    if it == niter - 1:
                        # final accumulation target is Gb, not S'
                        nc.vector.tensor_copy(ps[m][:], gb16[:, m, :])
                    else:
                        nc.vector.scalar_tensor_tensor(
                            ps[m][:], ps[m][:], THR, gb16[:, m, :],
                            Alu.min, Alu.add,
                        )
                if not last:
                    w_cur = neww

    nc.finalize()
    return nc


def _get_nc(niter):
    if niter not in _NC_CACHE:
        _NC_CACHE[niter] = _build(niter)
    return _NC_CACHE[niter]


def _prep_in_maps(Y, A):
    """Host precompute of the A-derived (voxel-independent) factor matrices,
    in float64: the inverse replaces the reference's Cholesky solve. Shards Y
    over voxels (transposed) and packs all device inputs into one
    pre-transposed [128, NPACK] fp16 array so every DMA descriptor is a
    multi-KB contiguous run."""
    A64 = A.astype(np.float64)
    LHS = A64.T @ A64 + RHO * np.eye(K_ATOMS)
    Minv = np.linalg.inv(LHS)
    Minv = (Minv + Minv.T) / 2
    Hm = A64 @ Minv  # [M, K]
    rsum = Minv.sum(axis=1)

    Ht = Hm.astype(np.float16)  # [M, K], M = 2*P exactly
    htp = Ht.reshape(2, P, K_ATOMS).transpose(1, 0, 2)  # [P, 2, K]
    Mi = Minv.astype(np.float16)
    mip = Mi.reshape(KB, P, K_ATOMS).transpose(1, 0, 2).reshape(P, KB * K_ATOMS)
    cneg = (-THR * rsum).astype(np.float16).reshape(KB, P).T  # [P, KB]
    fixed = np.concatenate([cneg, mip], axis=1)  # [P, KB + KB*K]

    in_maps = []
    for c in range(N_CORES):
        Yt = Y[c * BS : (c + 1) * BS, :].T.astype(np.float16)  # [M, BS]
        ytp = Yt.reshape(2, P, BS).transpose(1, 0, 2)  # [P, 2, BS]
        hy = np.concatenate([htp, ytp], axis=2).reshape(P, 2 * (K_ATOMS + BS))
        pk = np.ascontiguousarray(np.concatenate([hy, fixed], axis=1))
        in_maps.append({"packed": pk})
    return in_maps


def kernel(Y, A, max_iter):
    from concourse.bass_utils import run_bass_kernel_spmd

    Y = np.ascontiguousarray(np.asarray(Y, dtype=np.float32))
    A = np.ascontiguousarray(np.asarray(A, dtype=np.float32))
    niter = int(max_iter)
    assert Y.shape == (B_VOX, M_MEAS) and A.shape == (M_MEAS, K_ATOMS)
    if niter < 1:
        # zero-length scan returns the zero initial state
        return np.zeros((B_VOX, K_ATOMS), np.float32)

    in_maps = _prep_in_maps(Y, A)
    nc = _get_nc(niter)
    res = run_bass_kernel_spmd(nc, in_maps, core_ids=list(range(N_CORES)))

    outp = np.empty((B_VOX, K_ATOMS), np.float32)
    for c in range(N_CORES):
        outp[c * BS : (c + 1) * BS] = res.results[c]["out"].T.astype(np.float32)
    return outp
